# revision 1
# baseline (speedup 1.0000x reference)
"""Titans NeuralMemory forward on 8 Trainium2 NeuronCores.

Decomposition (validated vs reference in fp64/numpy):
  - Per-chunk MLP-loss gradients are rank-16: g_i(s) = l_i(s)^T r_i(s) with
    l/r factors [16, 256] from a batched forward/backward pass with the
    shared base weights.
  - The two associative scans have scalar per-chunk coefficients, so their
    composition is a lower-triangular [64, 64] matrix T = L_D @ L_A built
    stably via exp of cumulative log-sigmoid differences.
  - Retrieval never materializes fast weights: per layer,
      X_{i+1} = silu(X_i @ W_i + (X_i @ L_i^T * M) @ R_i),
    where M[r, j] = T[chunk(r), chunk(j)] expands T blockwise.

Sharding: 8 cores = 2 batch rows x 4 retrieve row-groups of 256 rows.
Each core redundantly runs the store phase for its batch row and computes
its own 256 retrieve rows; no collectives. Matmuls in fp32r (full PE rate).
"""
import os
import numpy as np

import concourse.bass as bass
import concourse.tile as tile
from concourse import bacc, mybir
from concourse.bass_utils import run_bass_kernel_spmd

AF = mybir.ActivationFunctionType
ALU = mybir.AluOpType
FP32 = mybir.dt.float32
FP32R = mybir.dt.float32r

B, L, D, C, DEPTH = 2, 1024, 256, 16, 4
N = L // C          # 64 chunks
P = 128
EPS = 1.1920929e-07
NCORES = 8
GROUPS = 4
RT = L // GROUPS    # 256 retrieve rows per core

# weight-blob layout (fp32r, per-partition fp32 word offsets)
WQ_O, WKV_O = 0, 512
W_O = WKV_O + 1024
WT_O = W_O + 2048
WP_O = WT_O + 1536
IDR_O = WP_O + 8
UT_O = IDR_O + 128
NUT_O = UT_O + 128
SEL_O = NUT_O + 128
WTS_SZ = SEL_O + 16

_CACHE = {}
LAST_PERF = {}


def _install_ntff_hook():
    """The agent image's antenv lacks axon_hooks; synthesize it so
    run_bass_kernel_spmd's trace=True path can reach the NTFF ctypes hook."""
    import sys
    import types
    try:
        from trn_agent_boot.trn_boot import _ntff_profile_via_ctypes
        hook = _ntff_profile_via_ctypes("/opt/axon/libaxon_pjrt.so")
    except Exception:
        return False
    if hook is None:
        return False
    mod = types.ModuleType("antenv.axon_hooks")
    mod.get_axon_ntff_profile_hook = lambda: hook
    mod.set_axon_ntff_profile_hook = lambda h: None
    sys.modules["antenv.axon_hooks"] = mod
    return True


def _build():
    nc = bacc.Bacc("TRN2", target_bir_lowering=False)

    seq_b = nc.dram_tensor("seq_b", [L, D], FP32, kind="ExternalInput")
    seq_q = nc.dram_tensor("seq_q", [RT, D], FP32, kind="ExternalInput")
    wts_d = nc.dram_tensor("wts_d", [P, WTS_SZ], FP32R, kind="ExternalInput")
    cst_d = nc.dram_tensor("cst_d", [P, 2 * N], FP32, kind="ExternalInput")
    out_d = nc.dram_tensor("out", [RT, D], FP32, kind="ExternalOutput")

    with tile.TileContext(nc) as tc:
        with (
            tc.tile_pool(name="big", bufs=1) as big,
            tc.tile_pool(name="rot", bufs=3) as rot,
            tc.tile_pool(name="pmm", bufs=2, space="PSUM") as pmm,
            tc.tile_pool(name="psc", bufs=2, space="PSUM") as psc,
            tc.tile_pool(name="ptr", bufs=2, space="PSUM") as ptr,
            tc.tile_pool(name="dram", bufs=1, space="DRAM") as dram,
        ):
            # ---------------- bulk loads ----------------
            wts = big.tile([P, WTS_SZ], FP32R)
            nc.sync.dma_start(wts, wts_d[:])
            cst = big.tile([P, 2 * N], FP32)
            nc.sync.dma_start(cst, cst_d[:])
            sq8 = big.tile([P, 8, D], FP32, tag="sq8")
            nc.sync.dma_start(sq8, seq_b[:].rearrange("(i p) d -> p i d", p=P))
            qs2 = big.tile([P, 2, D], FP32, tag="qs2")
            nc.sync.dma_start(qs2, seq_q[:].rearrange("(i p) d -> p i d", p=P))

            wq_sb = wts[:, WQ_O:WQ_O + 512].rearrange("p (k m) -> p k m", k=2)
            wkv_sb = wts[:, WKV_O:WKV_O + 1024].rearrange("p (k m) -> p k m", k=2)
            w_sb = wts[:, W_O:W_O + 2048].rearrange(
                "p (l k m) -> p l k m", l=4, k=2)
            wt_sb = wts[:, WT_O:WT_O + 1536].rearrange(
                "p (l k m) -> p l k m", l=3, k=2)
            wp_sb = wts[:, WP_O:WP_O + 8].rearrange("p (k m) -> p k m", k=2)
            identR = wts[:, IDR_O:IDR_O + 128]
            ut_sb = wts[:, UT_O:UT_O + 128]
            nut_sb = wts[:, NUT_O:NUT_O + 128]
            sel_sb = wts[:, SEL_O:SEL_O + 16]
            mls_sb = cst[:, 0:N]
            mut_sb = cst[:, N:2 * N]

            eps_sb = big.tile([P, 1], FP32)
            nc.vector.memset(eps_sb, EPS)

            # ---------------- rmsnorms (store + retrieve-q) ----------------
            def rmsnorm_make(x, tag):
                scr_a = rot.tile([P, D], FP32, tag="rms_scr", bufs=2)
                ms = rot.tile([P, 1], FP32, tag=f"{tag}ms", bufs=2)
                nc.scalar.activation(scr_a, x, AF.Square, accum_out=ms)
                lnv = rot.tile([P, 1], FP32, tag=f"{tag}ln", bufs=2)
                nc.scalar.activation(lnv, ms, AF.Ln, scale=1.0 / D, bias=eps_sb)
                rstd = rot.tile([P, 1], FP32, tag=f"{tag}rs", bufs=2)
                nc.scalar.activation(rstd, lnv, AF.Exp, scale=-0.5)
                out = rot.tile([P, D], FP32R, tag=f"{tag}o", bufs=4 if tag == "sn" else 2)
                nc.vector.tensor_scalar_mul(out, x, rstd)
                return out

            sn = [rmsnorm_make(sq8[:, i, :], "sn") for i in range(8)]
            rq = [rmsnorm_make(qs2[:, i, :], "rq") for i in range(2)]

            # ---------------- transposes: snT, rqT ----------------
            snT = [big.tile([P, L], FP32R, name=f"snT{k}", tag=f"snT{k}")
                   for k in range(2)]
            for grp in range(2):
                for ko in range(2):
                    tp = ptr.tile([P, 512], FP32R, tag="tr")
                    for ii in range(4):
                        i = grp * 4 + ii
                        nc.tensor.transpose(
                            tp[:, ii * P:(ii + 1) * P],
                            sn[i][:, ko * P:(ko + 1) * P], identR)
                    nc.vector.tensor_copy(
                        snT[ko][:, grp * 512:(grp + 1) * 512], tp)
            rqT = [big.tile([P, RT], FP32R, name=f"rqT{k}") for k in range(2)]
            for ko in range(2):
                tp = ptr.tile([P, 512], FP32R, tag="tr")
                for rt in range(2):
                    nc.tensor.transpose(
                        tp[:, rt * P:(rt + 1) * P],
                        rq[rt][:, ko * P:(ko + 1) * P], identR)
                nc.vector.tensor_copy(rqT[ko], tp[:, 0:RT])

            # ---------------- chunk sums -> T pipeline ----------------
            cmT = big.tile([P, 2, N], FP32R)
            with nc.allow_low_precision(reason="fp32r rounding of fp32 accum"):
                for ko in range(2):
                    nc.vector.reduce_sum(
                        cmT[:, ko, :],
                        snT[ko].rearrange("p (n c) -> p n c", c=C),
                        axis=mybir.AxisListType.X)

            zp = ptr.tile([N, 4], FP32, tag="tr")
            for ko in range(2):
                nc.tensor.matmul(zp, cmT[:, ko, :], wp_sb[:, ko, :],
                                 start=(ko == 0), stop=(ko == 1))
            # sigmoids first (one table), then ln/exp cluster
            sg = big.tile([P, 3], FP32)
            nc.vector.memset(sg, 0.0)
            nc.scalar.activation(sg[:N, 0:1], zp[:, 1:2], AF.Sigmoid)
            nc.scalar.activation(sg[:N, 1:2], zp[:, 2:3], AF.Sigmoid, scale=-1.0)
            nc.scalar.activation(sg[:N, 2:3], zp[:, 0:1], AF.Sigmoid)
            lg = big.tile([P, 3], FP32)
            nc.vector.memset(lg, 0.0)
            nc.scalar.activation(lg[:N, :], sg[:N, :], AF.Ln)
            lgr = big.tile([P, 2], FP32R)
            nc.vector.tensor_copy(lgr, lg[:, 0:2])
            cacc_p = ptr.tile([P, 2], FP32, tag="tr")
            nc.tensor.matmul(cacc_p, ut_sb, lgr, start=True, stop=True)
            cacc = big.tile([P, 2], FP32)
            nc.vector.tensor_copy(cacc, cacc_p)
            nacc_p = ptr.tile([P, 2], FP32, tag="tr")
            nc.tensor.matmul(nacc_p, nut_sb, lgr, start=True, stop=True)
            nacc = big.tile([P, 2], FP32)
            nc.vector.tensor_copy(nacc, nacc_p)

            # stage [NACC0 + ln(2 lr / D) | CACC1] -> DRAM -> row-bcasts.
            # Folding the surprise scale (2/D)*lr_s into T's s-columns lets
            # gg3 = v - pred with no broadcast dependency.
            stage = big.tile([P, 2], FP32)
            nc.vector.scalar_tensor_tensor(
                out=stage[:, 0:1], in0=nacc[:, 0:1],
                scalar=float(np.log(2.0 / D)), in1=lg[:, 2:3],
                op0=ALU.add, op1=ALU.add)
            nc.vector.tensor_copy(stage[:, 1:2], cacc[:, 1:2])
            scr = dram.tile([P, 2], FP32)
            nc.sync.dma_start(scr, stage)
            bc3 = big.tile([P, 2, N], FP32)
            for k in range(2):
                nc.sync.dma_start(bc3[:, k, :], bass.AP(
                    tensor=scr.tensor, offset=scr.offset + k,
                    ap=[[0, P], [2, N]]))
            ncarow = bc3[:, 0, :]
            pcdrow = bc3[:, 1, :]

            la = big.tile([P, N], FP32R)
            tmp1 = big.tile([P, N], FP32)
            nc.vector.scalar_tensor_tensor(
                out=tmp1, in0=ncarow, scalar=cacc[:, 0:1], in1=mls_sb,
                op0=ALU.add, op1=ALU.add)
            nc.scalar.activation(la, tmp1, AF.Exp)
            ldt = big.tile([P, N], FP32R)
            tmp2 = big.tile([P, N], FP32)
            nc.vector.scalar_tensor_tensor(
                out=tmp2, in0=pcdrow, scalar=nacc[:, 1:2], in1=mut_sb,
                op0=ALU.add, op1=ALU.add)
            nc.scalar.activation(ldt, tmp2, AF.Exp)

            tt_p = ptr.tile([N, N], FP32, tag="tr")
            nc.tensor.matmul(tt_p, ldt, la, start=True, stop=True)
            ttile = big.tile([P, N], FP32)
            nc.vector.memset(ttile, 0.0)
            nc.vector.tensor_copy(ttile[:N], tt_p)

            # maskbx_k[j, r] = T[toff + r//16, s(j)]  (expanded x16 in r)
            maskbx = []
            for k in range(8):
                ttx = rot.tile([P, P], FP32R, tag="ttx", bufs=2)
                nc.gpsimd.tensor_copy(
                    ttx[:N],
                    ttile[:N, k * 8:(k + 1) * 8, None].to_broadcast([N, 8, C]))
                mb_p = ptr.tile([P, C], FP32, tag="tr")
                nc.tensor.matmul(mb_p, ttx[:N], sel_sb[:N], start=True,
                                 stop=True)
                mb = rot.tile([P, C], FP32, tag="mb", bufs=2)
                nc.vector.tensor_copy(mb, mb_p)
                mbx = big.tile([P, RT], FP32, name=f"maskbx{k}")
                nc.gpsimd.tensor_copy(
                    mbx.rearrange("p (n c) -> p n c", c=C),
                    mb[:, :, None].to_broadcast([P, C, C]))
                maskbx.append(mbx)

            # ---------------- kv projection ----------------
            kT = [big.tile([P, L], FP32R, name=f"kT{k}") for k in range(2)]
            vT = [big.tile([P, L], FP32, name=f"vT{k}") for k in range(2)]
            for ko4 in range(4):
                dest = kT[ko4] if ko4 < 2 else vT[ko4 - 2]
                for rc in range(2):
                    sl = slice(rc * 512, (rc + 1) * 512)
                    mm = pmm.tile([P, 512], FP32, tag="mm")
                    for ki in range(2):
                        nc.tensor.matmul(
                            mm, wkv_sb[:, ki, ko4 * P:(ko4 + 1) * P],
                            snT[ki][:, sl], start=(ki == 0), stop=(ki == 1))
                    nc.vector.tensor_copy(dest[:, sl], mm)

            # ---------------- forward MLP ----------------
            Lf = [kT]
            dsT = []
            for i in range(3):
                a_next = [big.tile([P, L], FP32R, name=f"aT{i+1}_{k}")
                          for k in range(2)]
                ds_i = [big.tile([P, L], FP32, name=f"dsT{i}_{k}")
                        for k in range(2)]
                for mo in range(2):
                    for rc in range(2):
                        sl = slice(rc * 512, (rc + 1) * 512)
                        mm = pmm.tile([P, 512], FP32, tag="mm")
                        for ki in range(2):
                            nc.tensor.matmul(
                                mm, w_sb[:, i, ki, mo * P:(mo + 1) * P],
                                Lf[i][ki][:, sl],
                                start=(ki == 0), stop=(ki == 1))
                        sgt = rot.tile([P, 512], FP32, tag="sgt", bufs=2)
                        nc.scalar.activation(sgt, mm, AF.Sigmoid)
                        nc.vector.tensor_mul(a_next[mo][:, sl], mm, sgt)
                        # ds = sig * (1 + h - a); final mult off-path on gpsimd
                        t2 = rot.tile([P, 512], FP32, tag="t2", bufs=2)
                        nc.vector.scalar_tensor_tensor(
                            out=t2, in0=mm, scalar=1.0, in1=a_next[mo][:, sl],
                            op0=ALU.add, op1=ALU.subtract)
                        nc.gpsimd.tensor_mul(ds_i[mo][:, sl], sgt, t2)
                Lf.append(a_next)
                dsT.append(ds_i)

            # ---------------- pred + gg3 ----------------
            ggA = [big.tile([P, L], FP32R, name=f"ggA{k}", tag=f"snT{k}")
                   for k in range(2)]
            ggB = [big.tile([P, L], FP32R, name="ggB0", tag="sq8"),
                   big.tile([P, L], FP32R, name="ggB1", tag="qs2")]
            for mo in range(2):
                for rc in range(2):
                    sl = slice(rc * 512, (rc + 1) * 512)
                    mm = pmm.tile([P, 512], FP32, tag="mm")
                    for ki in range(2):
                        nc.tensor.matmul(
                            mm, w_sb[:, 3, ki, mo * P:(mo + 1) * P],
                            Lf[3][ki][:, sl], start=(ki == 0), stop=(ki == 1))
                    nc.vector.tensor_sub(ggA[mo][:, sl], vT[mo][:, sl], mm)

            # ---------------- R factors + backward ----------------
            Rf = {i: [big.tile([P, D], FP32R, name=f"Rf{i}_{jt}")
                      for jt in range(8)] for i in range(4)}

            def emit_R(layer, src):
                for jt in range(8):
                    tp = ptr.tile([P, 512], FP32R, tag="tr")
                    for mo in range(2):
                        nc.tensor.transpose(
                            tp[:, mo * P:(mo + 1) * P],
                            src[mo][:, jt * P:(jt + 1) * P], identR)
                    nc.vector.tensor_copy(Rf[layer][jt], tp[:, 0:D])

            emit_R(3, ggA)
            gg_cur, gg_next = ggA, ggB
            for i in (3, 2, 1):
                for mo in range(2):
                    for rc in range(2):
                        sl = slice(rc * 512, (rc + 1) * 512)
                        mm = pmm.tile([P, 512], FP32, tag="mm")
                        for ki in range(2):
                            nc.tensor.matmul(
                                mm, wt_sb[:, i - 1, ki, mo * P:(mo + 1) * P],
                                gg_cur[ki][:, sl],
                                start=(ki == 0), stop=(ki == 1))
                        nc.vector.tensor_mul(
                            gg_next[mo][:, sl], mm, dsT[i - 1][mo][:, sl])
                emit_R(i - 1, gg_next)
                gg_cur, gg_next = gg_next, gg_cur

            # ---------------- retrieve ----------------
            XTa = [big.tile([P, RT], FP32R, name=f"XTa{k}") for k in range(2)]
            XTb = [big.tile([P, RT], FP32R, name=f"XTb{k}") for k in range(2)]
            for mo in range(2):
                sc = psc.tile([P, RT], FP32, tag="sc")
                for ki in range(2):
                    nc.tensor.matmul(sc, wq_sb[:, ki, mo * P:(mo + 1) * P],
                                     rqT[ki], start=(ki == 0), stop=(ki == 1))
                nc.vector.tensor_copy(XTa[mo], sc)

            XTin, XTout = XTa, XTb
            X4T = [big.tile([P, RT], FP32R, name=f"X4T{k}") for k in range(2)]
            for i in range(4):
                msc = []
                for jt in range(8):
                    sc = psc.tile([P, RT], FP32, tag="sc")
                    for ki in range(2):
                        nc.tensor.matmul(
                            sc, Lf[i][ki][:, jt * P:(jt + 1) * P], XTin[ki],
                            start=(ki == 0), stop=(ki == 1))
                    m = rot.tile([P, RT], FP32R, tag="msc", bufs=8)
                    nc.vector.tensor_mul(m, sc, maskbx[jt])
                    msc.append(m)
                for mo in range(2):
                    y = psc.tile([P, RT], FP32, tag="y")
                    for ki in range(2):
                        nc.tensor.matmul(
                            y, w_sb[:, i, ki, mo * P:(mo + 1) * P], XTin[ki],
                            start=(ki == 0), stop=False)
                    for jt in range(8):
                        nc.tensor.matmul(
                            y, Rf[i][jt][:, mo * P:(mo + 1) * P], msc[jt],
                            start=False, stop=(jt == 7))
                    if i < 3:
                        sgt = rot.tile([P, RT], FP32, tag="sgr")
                        nc.scalar.activation(sgt, y, AF.Sigmoid)
                        nc.vector.tensor_mul(XTout[mo], y, sgt)
                    else:
                        nc.vector.tensor_copy(X4T[mo], y)
                XTin, XTout = XTout, XTin

            # ---------------- postnorm + output ----------------
            for rt in range(2):
                tp = ptr.tile([P, 512], FP32R, tag="tr")
                for mo in range(2):
                    nc.tensor.transpose(
                        tp[:, mo * P:(mo + 1) * P],
                        X4T[mo][:, rt * P:(rt + 1) * P], identR)
                x4 = rot.tile([P, D], FP32, tag="x4", bufs=2)
                nc.vector.tensor_copy(x4, tp[:, 0:D])
                scr_a = rot.tile([P, D], FP32, tag="rms_scr", bufs=2)
                ms = rot.tile([P, 1], FP32, tag="pms", bufs=2)
                nc.scalar.activation(scr_a, x4, AF.Square, accum_out=ms)
                lnv = rot.tile([P, 1], FP32, tag="pln", bufs=2)
                nc.scalar.activation(lnv, ms, AF.Ln, scale=1.0 / D, bias=eps_sb)
                rstd = rot.tile([P, 1], FP32, tag="prs", bufs=2)
                nc.scalar.activation(rstd, lnv, AF.Exp, scale=-0.5)
                o = rot.tile([P, D], FP32, tag="osb", bufs=2)
                nc.vector.tensor_scalar_mul(o, x4, rstd)
                nc.sync.dma_start(out_d[rt * P:(rt + 1) * P, :], o)

    nc.compile()
    return nc


def _host_prep(inputs):
    seq = np.ascontiguousarray(np.asarray(inputs["seq"], dtype=np.float32))
    Wq = np.asarray(inputs["Wq"], dtype=np.float32)
    Wkv = np.asarray(inputs["Wkv"], dtype=np.float32)
    Ws = [np.asarray(inputs[f"W{i}"], dtype=np.float32) for i in range(4)]
    wa = np.asarray(inputs["w_adapt"], dtype=np.float32)
    wm = np.asarray(inputs["w_mom"], dtype=np.float32)
    wd = np.asarray(inputs["w_decay"], dtype=np.float32)

    def kxm(w):  # [K, M] -> [128, (K/128)*M]
        return w.reshape(w.shape[0] // P, P, w.shape[1]).transpose(1, 0, 2) \
            .reshape(P, -1)

    ii = np.arange(N)
    tri = np.triu(np.ones((N, N), np.float32))
    wpack = np.zeros((D, 4), np.float32)
    wpack[:, 0] = wa
    wpack[:, 1] = wm
    wpack[:, 2] = wd
    wpack *= (1.0 / C)

    wts = np.zeros((P, WTS_SZ), np.float32)
    wts[:, WQ_O:WQ_O + 512] = kxm(Wq)
    wts[:, WKV_O:WKV_O + 1024] = kxm(Wkv)
    w_all = np.stack(Ws).reshape(4, 2, P, D).transpose(2, 0, 1, 3)
    wts[:, W_O:W_O + 2048] = w_all.reshape(P, -1)
    wt_all = np.stack([Ws[1].T, Ws[2].T, Ws[3].T]) \
        .reshape(3, 2, P, D).transpose(2, 0, 1, 3)
    wts[:, WT_O:WT_O + 1536] = wt_all.reshape(P, -1)
    wts[:, WP_O:WP_O + 8] = kxm(wpack)
    wts[:, IDR_O:IDR_O + 128] = np.eye(P, dtype=np.float32)
    wts[:N, UT_O:UT_O + N] = tri
    wts[:N, NUT_O:NUT_O + N] = -tri

    cst = np.full((P, 2 * N), -1e30, np.float32)
    cst[:N, 0:N] = np.where(ii[:, None] >= ii[None, :], 0.0, -1e30)
    cst[:N, N:2 * N] = np.where(ii[:, None] <= ii[None, :], 0.0, -1e30)

    in_maps = []
    for core in range(NCORES):
        b, g = divmod(core, GROUPS)
        wts_c = wts.copy()
        sel = np.zeros((P, C), np.float32)
        toff = C * g
        sel[toff:toff + C, :] = np.eye(C, dtype=np.float32)
        wts_c[:, SEL_O:SEL_O + C] = sel
        m = {"wts_d": wts_c, "cst_d": cst, "seq_b": seq[b]}
        qs = np.zeros((RT, D), np.float32)
        j0 = RT * g + (C - 1)
        src = seq[b, j0:min(j0 + RT, L)]
        qs[:len(src)] = src
        m["seq_q"] = qs
        in_maps.append(m)
    return in_maps


def kernel(**inputs):
    if "nc" not in _CACHE:
        _CACHE["nc"] = _build()
    nc = _CACHE["nc"]
    in_maps = _host_prep(inputs)
    trace = bool(int(os.environ.get("KERNEL_TRACE", "0")))
    if trace:
        try:
            from antenv.axon_hooks import get_axon_ntff_profile_hook  # noqa: F401
        except ImportError:
            trace = _install_ntff_hook()
    res = run_bass_kernel_spmd(
        nc, in_maps, core_ids=list(range(NCORES)), trace=trace)
    LAST_PERF.clear()
    LAST_PERF.update(dict(
        exec_time_ns=res.exec_time_ns,
        mean_exec_time_ns=res.mean_exec_time_ns,
        profile_json=res.profile_json,
        trace=res.instructions_and_trace[1] if res.instructions_and_trace else None,
    ))
    final = np.zeros((B, L, D), np.float32)
    for core in range(NCORES):
        b, g = divmod(core, GROUPS)
        j0 = RT * g + (C - 1)
        n = min(RT, L - j0)
        final[b, j0:j0 + n] = res.results[core]["out"][:n]
    return final



# revision 11
# speedup vs baseline: 1.8983x; 1.8983x over previous
"""Titans NeuralMemory forward on 8 Trainium2 NeuronCores.

Decomposition (validated vs reference in fp64/numpy):
  - Per-chunk MLP-loss gradients are rank-16: g_i(s) = l_i(s)^T r_i(s) with
    l/r factors [16, 256] from a batched forward/backward pass with the
    shared base weights.
  - The two associative scans have scalar per-chunk coefficients, so their
    composition is a lower-triangular [64, 64] matrix T = L_D @ L_A built
    stably via exp of cumulative log-sigmoid differences.
  - Retrieval never materializes fast weights: per layer,
      X_{i+1} = silu(X_i @ W_i + (X_i @ L_i^T * M) @ R_i),
    where M[r, j] = T[chunk(r), chunk(j)] expands T blockwise.

Sharding: 8 cores = 2 batch rows x 4 retrieve row-groups of 256 rows.
Each core redundantly runs the store phase for its batch row and computes
its own 256 retrieve rows; no collectives. Matmuls in fp32r (full PE rate).

v2 perf restructure vs v1:
  - The [64]-chunk cumulative-gate row broadcast is done on-chip with a
    PE transpose + K=2 ones-matmul instead of a DRAM round-trip whose
    4-byte-element DMAs took ~40us each and stalled the engine FIFOs.
  - Scalar activations are emitted grouped by activation table
    (ln/exp -> sigmoid -> ln/exp -> silu -> ln/exp, with table-free
    Square/Copy in between): 5 ACT_TABLE_LOADs instead of 28.
  - Retrieve silu is a single scalar Silu op (no sigmoid+vector mul).
  - PSUM->SBUF copies are split between scalar (ACT Copy) and vector.
  - Input DMAs reordered (seq first, small weight tail before big blob).
"""
import os
import numpy as np

import concourse.bass as bass
import concourse.tile as tile
from concourse import bacc, mybir
from concourse.bass_utils import run_bass_kernel_spmd

AF = mybir.ActivationFunctionType
ALU = mybir.AluOpType
FP32 = mybir.dt.float32
FP32R = mybir.dt.float32r

B, L, D, C, DEPTH = 2, 1024, 256, 16, 4
N = L // C          # 64 chunks
P = 128
EPS = 1.1920929e-07
NCORES = 8
GROUPS = 4
RT = L // GROUPS    # 256 retrieve rows per core

# weight-blob layout (fp32r, per-partition fp32 word offsets)
WQ_O, WKV_O = 0, 512
W_O = WKV_O + 1024
WT_O = W_O + 2048
WP_O = WT_O + 1536
IDR_O = WP_O + 8
UT_O = IDR_O + 128
NUT_O = UT_O + 128
SEL_O = NUT_O + 128
BSEL_O = SEL_O + 16
WTS_SZ = BSEL_O + 256

_CACHE = {}
LAST_PERF = {}


def _install_ntff_hook():
    """The agent image's antenv lacks axon_hooks; synthesize it so
    run_bass_kernel_spmd's trace=True path can reach the NTFF ctypes hook."""
    import sys
    import types
    try:
        from trn_agent_boot.trn_boot import _ntff_profile_via_ctypes
        hook = _ntff_profile_via_ctypes("/opt/axon/libaxon_pjrt.so")
    except Exception:
        return False
    if hook is None:
        return False
    mod = types.ModuleType("antenv.axon_hooks")
    mod.get_axon_ntff_profile_hook = lambda: hook
    mod.set_axon_ntff_profile_hook = lambda h: None
    sys.modules["antenv.axon_hooks"] = mod
    return True


def _build():
    nc = bacc.Bacc("TRN2", target_bir_lowering=False)

    seq_b = nc.dram_tensor("seq_b", [L, D], FP32, kind="ExternalInput")
    seq_q = nc.dram_tensor("seq_q", [RT, D], FP32, kind="ExternalInput")
    wts_d = nc.dram_tensor("wts_d", [P, WTS_SZ], FP32R, kind="ExternalInput")
    cst_d = nc.dram_tensor("cst_d", [P, 2 * N], FP32, kind="ExternalInput")
    out_d = nc.dram_tensor("out", [RT, D], FP32, kind="ExternalOutput")

    with tile.TileContext(nc) as tc:
        with (
            tc.tile_pool(name="big", bufs=1) as big,
            tc.tile_pool(name="rot", bufs=3) as rot,
            tc.tile_pool(name="pmm", bufs=2, space="PSUM") as pmm,
            tc.tile_pool(name="psc", bufs=2, space="PSUM") as psc,
            tc.tile_pool(name="ptr", bufs=2, space="PSUM") as ptr,
        ):
            # ---------------- bulk loads (seq first, identity/tri next) ----
            sq8 = big.tile([P, 8, D], FP32, tag="sq8")
            nc.sync.dma_start(sq8, seq_b[:].rearrange("(i p) d -> p i d", p=P))
            qs2 = big.tile([P, 2, D], FP32, tag="qs2")
            nc.sync.dma_start(qs2, seq_q[:].rearrange("(i p) d -> p i d", p=P))
            wts = big.tile([P, WTS_SZ], FP32R)
            nc.sync.dma_start(wts[:, IDR_O:WTS_SZ], wts_d[:, IDR_O:WTS_SZ])
            nc.sync.dma_start(wts[:, 0:IDR_O], wts_d[:, 0:IDR_O])
            cst = big.tile([P, 2 * N], FP32)
            nc.sync.dma_start(cst, cst_d[:])

            wq_sb = wts[:, WQ_O:WQ_O + 512].rearrange("p (k m) -> p k m", k=2)
            wkv_sb = wts[:, WKV_O:WKV_O + 1024].rearrange("p (k m) -> p k m", k=2)
            w_sb = wts[:, W_O:W_O + 2048].rearrange(
                "p (l k m) -> p l k m", l=4, k=2)
            wt_sb = wts[:, WT_O:WT_O + 1536].rearrange(
                "p (l k m) -> p l k m", l=3, k=2)
            wp_sb = wts[:, WP_O:WP_O + 8].rearrange("p (k m) -> p k m", k=2)
            identR = wts[:, IDR_O:IDR_O + 128]
            ut_sb = wts[:, UT_O:UT_O + 128]
            nut_sb = wts[:, NUT_O:NUT_O + 128]
            sel_sb = wts[:, SEL_O:SEL_O + 16]
            bsel_sb = wts[:, BSEL_O:BSEL_O + 256]
            mls_sb = cst[:, 0:N]
            mut_sb = cst[:, N:2 * N]

            eps_sb = big.tile([P, 1], FP32)
            nc.vector.memset(eps_sb, EPS)
            # pin the ln/exp act table before the Squares pick another set
            dummy = big.tile([P, 1], FP32)
            nc.scalar.activation(dummy, eps_sb, AF.Ln)

            # ---------------- rmsnorms, phased by activation fn ----------
            xs = [sq8[:, i, :] for i in range(8)] + [qs2[:, i, :] for i in range(2)]
            msb = big.tile([P, 10], FP32)
            for i, x in enumerate(xs):
                scr_a = rot.tile([P, D], FP32, tag="rms_scr", bufs=2)
                nc.scalar.activation(scr_a, x, AF.Square, accum_out=msb[:, i:i + 1])
            lnv = big.tile([P, 10], FP32)
            nc.scalar.activation(lnv, msb, AF.Ln, scale=1.0 / D, bias=eps_sb)
            rstd = big.tile([P, 10], FP32)
            nc.scalar.activation(rstd, lnv, AF.Exp, scale=-0.5)
            sn = [big.tile([P, D], FP32R, name=f"sn{i}") for i in range(8)]
            rq = [big.tile([P, D], FP32R, name=f"rq{i}") for i in range(2)]
            for i in range(8):
                nc.vector.tensor_scalar_mul(sn[i], xs[i], rstd[:, i:i + 1])
            for i in range(2):
                nc.vector.tensor_scalar_mul(rq[i], xs[8 + i], rstd[:, 8 + i:9 + i])

            # ---------------- transposes: snT, rqT ----------------
            snT = [big.tile([P, L], FP32R, name=f"snT{k}", tag=f"snT{k}")
                   for k in range(2)]
            for grp in range(2):
                for ko in range(2):
                    tp = ptr.tile([P, 512], FP32R, tag="tr")
                    for ii in range(4):
                        i = grp * 4 + ii
                        nc.tensor.transpose(
                            tp[:, ii * P:(ii + 1) * P],
                            sn[i][:, ko * P:(ko + 1) * P], identR)
                    eng = nc.vector if (grp + ko) % 2 == 0 else nc.scalar
                    if eng is nc.vector:
                        nc.vector.tensor_copy(
                            snT[ko][:, grp * 512:(grp + 1) * 512], tp)
                    else:
                        nc.scalar.copy(
                            snT[ko][:, grp * 512:(grp + 1) * 512], tp)
            rqT = [big.tile([P, RT], FP32R, name=f"rqT{k}") for k in range(2)]
            for ko in range(2):
                tp = ptr.tile([P, 512], FP32R, tag="tr")
                for rt in range(2):
                    nc.tensor.transpose(
                        tp[:, rt * P:(rt + 1) * P],
                        rq[rt][:, ko * P:(ko + 1) * P], identR)
                nc.scalar.copy(rqT[ko], tp[:, 0:RT])

            # ---------------- chunk sums -> gate logits ----------------
            cmT = big.tile([P, 2, N], FP32R)
            with nc.allow_low_precision(reason="fp32r rounding of fp32 accum"):
                for ko in range(2):
                    nc.vector.reduce_sum(
                        cmT[:, ko, :],
                        snT[ko].rearrange("p (n c) -> p n c", c=C),
                        axis=mybir.AxisListType.X)

            zp = ptr.tile([N, 4], FP32, tag="tr")
            for ko in range(2):
                nc.tensor.matmul(zp, cmT[:, ko, :], wp_sb[:, ko, :],
                                 start=(ko == 0), stop=(ko == 1))
            # gate sigmoids (head of the sigmoid-table block; fwd sigmoids
            # follow with only table-free Copies in between)
            sg = big.tile([P, 3], FP32)
            nc.vector.memset(sg, 0.0)
            nc.scalar.activation(sg[:N, 0:1], zp[:, 1:2], AF.Sigmoid)
            nc.scalar.activation(sg[:N, 1:2], zp[:, 2:3], AF.Sigmoid, scale=-1.0)
            nc.scalar.activation(sg[:N, 2:3], zp[:, 0:1], AF.Sigmoid)

            # ---------------- kv projection ----------------
            kT = [big.tile([P, L], FP32R, name=f"kT{k}") for k in range(2)]
            vT = [big.tile([P, L], FP32, name=f"vT{k}") for k in range(2)]
            for ko4 in range(4):
                dest = kT[ko4] if ko4 < 2 else vT[ko4 - 2]
                for rc in range(2):
                    sl = slice(rc * 512, (rc + 1) * 512)
                    mm = pmm.tile([P, 512], FP32, tag="mm")
                    for ki in range(2):
                        nc.tensor.matmul(
                            mm, wkv_sb[:, ki, ko4 * P:(ko4 + 1) * P],
                            snT[ki][:, sl], start=(ki == 0), stop=(ki == 1))
                    nc.scalar.copy(dest[:, sl], mm)

            # ---------------- forward MLP ----------------
            Lf = [kT]
            dsT = []
            for i in range(3):
                a_next = [big.tile([P, L], FP32R, name=f"aT{i+1}_{k}")
                          for k in range(2)]
                ds_i = [big.tile([P, L], FP32, name=f"dsT{i}_{k}")
                        for k in range(2)]
                for mo in range(2):
                    for rc in range(2):
                        sl = slice(rc * 512, (rc + 1) * 512)
                        mm = pmm.tile([P, 512], FP32, tag="mm")
                        for ki in range(2):
                            nc.tensor.matmul(
                                mm, w_sb[:, i, ki, mo * P:(mo + 1) * P],
                                Lf[i][ki][:, sl],
                                start=(ki == 0), stop=(ki == 1))
                        sgt = rot.tile([P, 512], FP32, tag="sgt", bufs=2)
                        nc.scalar.activation(sgt, mm, AF.Sigmoid)
                        nc.vector.tensor_mul(a_next[mo][:, sl], mm, sgt)
                        # ds = sig * (1 + h - a); final mult off-path on gpsimd
                        t2 = rot.tile([P, 512], FP32, tag="t2", bufs=2)
                        nc.vector.scalar_tensor_tensor(
                            out=t2, in0=mm, scalar=1.0, in1=a_next[mo][:, sl],
                            op0=ALU.add, op1=ALU.subtract)
                        nc.gpsimd.tensor_mul(ds_i[mo][:, sl], sgt, t2)
                Lf.append(a_next)
                dsT.append(ds_i)

            # ---------------- pred + gg3 ----------------
            ggA = [big.tile([P, L], FP32R, name=f"ggA{k}", tag=f"snT{k}")
                   for k in range(2)]
            ggB = [big.tile([P, L], FP32R, name="ggB0", tag="sq8"),
                   big.tile([P, L], FP32R, name="ggB1", tag="qs2")]
            for mo in range(2):
                for rc in range(2):
                    sl = slice(rc * 512, (rc + 1) * 512)
                    mm = pmm.tile([P, 512], FP32, tag="mm")
                    for ki in range(2):
                        nc.tensor.matmul(
                            mm, w_sb[:, 3, ki, mo * P:(mo + 1) * P],
                            Lf[3][ki][:, sl], start=(ki == 0), stop=(ki == 1))
                    nc.vector.tensor_sub(ggA[mo][:, sl], vT[mo][:, sl], mm)

            # ---------------- T pipeline (gates -> masks), on-chip ------
            # scalar block: ln of gate sigmoids, then the two exps (same
            # natural_log_exp table, after all fwd sigmoids in the stream)
            lg = big.tile([P, 3], FP32)
            nc.vector.memset(lg, 0.0)
            nc.scalar.activation(lg[:N, :], sg[:N, :], AF.Ln)
            lgr = big.tile([P, 2], FP32R)
            nc.vector.tensor_copy(lgr, lg[:, 0:2])
            cacc_p = ptr.tile([P, 2], FP32, tag="tr")
            nc.tensor.matmul(cacc_p, ut_sb, lgr, start=True, stop=True)
            cacc = big.tile([P, 2], FP32)
            nc.vector.tensor_copy(cacc, cacc_p)
            nacc_p = ptr.tile([P, 2], FP32, tag="tr")
            nc.tensor.matmul(nacc_p, nut_sb, lgr, start=True, stop=True)
            nacc = big.tile([P, 2], FP32)
            nc.vector.tensor_copy(nacc, nacc_p)

            # stage [NACC0 + ln(2 lr / D) | CACC1]; transpose to a 2-row
            # tile and broadcast across partitions with a K=2 ones-matmul.
            # Folding the surprise scale (2/D)*lr_s into T's s-columns lets
            # gg3 = v - pred with no broadcast dependency.
            stage = big.tile([P, 2], FP32)
            nc.vector.scalar_tensor_tensor(
                out=stage[:, 0:1], in0=nacc[:, 0:1],
                scalar=float(np.log(2.0 / D)), in1=lg[:, 2:3],
                op0=ALU.add, op1=ALU.add)
            nc.vector.tensor_copy(stage[:, 1:2], cacc[:, 1:2])
            stg128 = big.tile([P, P], FP32)
            nc.vector.memset(stg128, 0.0)
            nc.vector.tensor_copy(stg128[:, 0:2], stage)
            stg128r = big.tile([P, P], FP32R)
            nc.vector.tensor_copy(stg128r, stg128)
            tp2 = ptr.tile([P, 512], FP32R, tag="tr")
            nc.tensor.transpose(tp2[:, 0:P], stg128r, identR)
            st_sb = big.tile([P, P], FP32R)
            nc.vector.tensor_copy(st_sb, tp2[:, 0:P])
            bc = ptr.tile([P, 2, N], FP32, tag="tr")
            nc.tensor.matmul(bc[:, 0, :], bsel_sb[:, 0:P], st_sb[:, 0:N],
                             start=True, stop=True)
            nc.tensor.matmul(bc[:, 1, :], bsel_sb[:, P:2 * P], st_sb[:, 0:N],
                             start=True, stop=True)

            la = big.tile([P, N], FP32R)
            tmp1 = big.tile([P, N], FP32)
            nc.vector.scalar_tensor_tensor(
                out=tmp1, in0=bc[:, 0, :], scalar=cacc[:, 0:1], in1=mls_sb,
                op0=ALU.add, op1=ALU.add)
            nc.scalar.activation(la, tmp1, AF.Exp)
            ldt = big.tile([P, N], FP32R)
            tmp2 = big.tile([P, N], FP32)
            nc.vector.scalar_tensor_tensor(
                out=tmp2, in0=bc[:, 1, :], scalar=nacc[:, 1:2], in1=mut_sb,
                op0=ALU.add, op1=ALU.add)
            nc.scalar.activation(ldt, tmp2, AF.Exp)

            tt_p = ptr.tile([N, N], FP32, tag="tr")
            nc.tensor.matmul(tt_p, ldt, la, start=True, stop=True)
            ttile = big.tile([P, N], FP32)
            nc.vector.memset(ttile, 0.0)
            nc.vector.tensor_copy(ttile[:N], tt_p)

            # maskbx_k[j, r] = T[toff + r//16, s(j)]  (expanded x16 in r)
            maskbx = []
            for k in range(8):
                ttx = rot.tile([P, P], FP32R, tag="ttx", bufs=2)
                nc.gpsimd.tensor_copy(
                    ttx[:N],
                    ttile[:N, k * 8:(k + 1) * 8, None].to_broadcast([N, 8, C]))
                mb_p = ptr.tile([P, C], FP32, tag="tr")
                nc.tensor.matmul(mb_p, ttx[:N], sel_sb[:N], start=True,
                                 stop=True)
                mb = rot.tile([P, C], FP32, tag="mb", bufs=2)
                nc.vector.tensor_copy(mb, mb_p)
                mbx = big.tile([P, RT], FP32, name=f"maskbx{k}")
                nc.gpsimd.tensor_copy(
                    mbx.rearrange("p (n c) -> p n c", c=C),
                    mb[:, :, None].to_broadcast([P, C, C]))
                maskbx.append(mbx)

            # ---------------- R factors + backward ----------------
            Rf = {i: [big.tile([P, D], FP32R, name=f"Rf{i}_{jt}")
                      for jt in range(8)] for i in range(4)}

            def emit_R(layer, src):
                for jt in range(8):
                    tp = ptr.tile([P, 512], FP32R, tag="tr")
                    for mo in range(2):
                        nc.tensor.transpose(
                            tp[:, mo * P:(mo + 1) * P],
                            src[mo][:, jt * P:(jt + 1) * P], identR)
                    if jt % 2 == 0:
                        nc.vector.tensor_copy(Rf[layer][jt], tp[:, 0:D])
                    else:
                        nc.scalar.copy(Rf[layer][jt], tp[:, 0:D])

            emit_R(3, ggA)
            gg_cur, gg_next = ggA, ggB
            for i in (3, 2, 1):
                for mo in range(2):
                    for rc in range(2):
                        sl = slice(rc * 512, (rc + 1) * 512)
                        mm = pmm.tile([P, 512], FP32, tag="mm")
                        for ki in range(2):
                            nc.tensor.matmul(
                                mm, wt_sb[:, i - 1, ki, mo * P:(mo + 1) * P],
                                gg_cur[ki][:, sl],
                                start=(ki == 0), stop=(ki == 1))
                        nc.vector.tensor_mul(
                            gg_next[mo][:, sl], mm, dsT[i - 1][mo][:, sl])
                emit_R(i - 1, gg_next)
                gg_cur, gg_next = gg_next, gg_cur

            # ---------------- retrieve ----------------
            XTa = [big.tile([P, RT], FP32R, name=f"XTa{k}") for k in range(2)]
            XTb = [big.tile([P, RT], FP32R, name=f"XTb{k}") for k in range(2)]
            for mo in range(2):
                sc = psc.tile([P, RT], FP32, tag="sc")
                for ki in range(2):
                    nc.tensor.matmul(sc, wq_sb[:, ki, mo * P:(mo + 1) * P],
                                     rqT[ki], start=(ki == 0), stop=(ki == 1))
                nc.vector.tensor_copy(XTa[mo], sc)

            XTin, XTout = XTa, XTb
            X4T = [big.tile([P, RT], FP32R, name=f"X4T{k}") for k in range(2)]
            for i in range(4):
                msc = []
                for jt in range(8):
                    sc = psc.tile([P, RT], FP32, tag="sc")
                    for ki in range(2):
                        nc.tensor.matmul(
                            sc, Lf[i][ki][:, jt * P:(jt + 1) * P], XTin[ki],
                            start=(ki == 0), stop=(ki == 1))
                    m = rot.tile([P, RT], FP32R, tag="msc", bufs=8)
                    nc.vector.tensor_mul(m, sc, maskbx[jt])
                    msc.append(m)
                for mo in range(2):
                    y = psc.tile([P, RT], FP32, tag="y")
                    for ki in range(2):
                        nc.tensor.matmul(
                            y, w_sb[:, i, ki, mo * P:(mo + 1) * P], XTin[ki],
                            start=(ki == 0), stop=False)
                    for jt in range(8):
                        nc.tensor.matmul(
                            y, Rf[i][jt][:, mo * P:(mo + 1) * P], msc[jt],
                            start=False, stop=(jt == 7))
                    if i < 3:
                        nc.scalar.activation(XTout[mo], y, AF.Silu)
                    else:
                        nc.vector.tensor_copy(X4T[mo], y)
                XTin, XTout = XTout, XTin

            # ---------------- postnorm + output ----------------
            x4 = []
            pms = big.tile([P, 2], FP32)
            for rt in range(2):
                tp = ptr.tile([P, 512], FP32R, tag="tr")
                for mo in range(2):
                    nc.tensor.transpose(
                        tp[:, mo * P:(mo + 1) * P],
                        X4T[mo][:, rt * P:(rt + 1) * P], identR)
                x4t = rot.tile([P, D], FP32, tag="x4", bufs=2)
                nc.vector.tensor_copy(x4t, tp[:, 0:D])
                x4.append(x4t)
                scr_a = rot.tile([P, D], FP32, tag="rms_scr", bufs=2)
                nc.scalar.activation(scr_a, x4t, AF.Square,
                                     accum_out=pms[:, rt:rt + 1])
            pln = big.tile([P, 2], FP32)
            nc.scalar.activation(pln, pms, AF.Ln, scale=1.0 / D, bias=eps_sb)
            prs = big.tile([P, 2], FP32)
            nc.scalar.activation(prs, pln, AF.Exp, scale=-0.5)
            for rt in range(2):
                o = rot.tile([P, D], FP32, tag="osb", bufs=2)
                nc.vector.tensor_scalar_mul(o, x4[rt], prs[:, rt:rt + 1])
                nc.sync.dma_start(out_d[rt * P:(rt + 1) * P, :], o)

    nc.compile()
    return nc


def _host_prep(inputs):
    seq = np.ascontiguousarray(np.asarray(inputs["seq"], dtype=np.float32))
    Wq = np.asarray(inputs["Wq"], dtype=np.float32)
    Wkv = np.asarray(inputs["Wkv"], dtype=np.float32)
    Ws = [np.asarray(inputs[f"W{i}"], dtype=np.float32) for i in range(4)]
    wa = np.asarray(inputs["w_adapt"], dtype=np.float32)
    wm = np.asarray(inputs["w_mom"], dtype=np.float32)
    wd = np.asarray(inputs["w_decay"], dtype=np.float32)

    def kxm(w):  # [K, M] -> [128, (K/128)*M]
        return w.reshape(w.shape[0] // P, P, w.shape[1]).transpose(1, 0, 2) \
            .reshape(P, -1)

    ii = np.arange(N)
    tri = np.triu(np.ones((N, N), np.float32))
    wpack = np.zeros((D, 4), np.float32)
    wpack[:, 0] = wa
    wpack[:, 1] = wm
    wpack[:, 2] = wd
    wpack *= (1.0 / C)

    wts = np.zeros((P, WTS_SZ), np.float32)
    wts[:, WQ_O:WQ_O + 512] = kxm(Wq)
    wts[:, WKV_O:WKV_O + 1024] = kxm(Wkv)
    w_all = np.stack(Ws).reshape(4, 2, P, D).transpose(2, 0, 1, 3)
    wts[:, W_O:W_O + 2048] = w_all.reshape(P, -1)
    wt_all = np.stack([Ws[1].T, Ws[2].T, Ws[3].T]) \
        .reshape(3, 2, P, D).transpose(2, 0, 1, 3)
    wts[:, WT_O:WT_O + 1536] = wt_all.reshape(P, -1)
    wts[:, WP_O:WP_O + 8] = kxm(wpack)
    wts[:, IDR_O:IDR_O + 128] = np.eye(P, dtype=np.float32)
    wts[:N, UT_O:UT_O + N] = tri
    wts[:N, NUT_O:NUT_O + N] = -tri
    wts[0, BSEL_O:BSEL_O + P] = 1.0
    wts[1, BSEL_O + P:BSEL_O + 2 * P] = 1.0

    cst = np.full((P, 2 * N), -1e30, np.float32)
    cst[:N, 0:N] = np.where(ii[:, None] >= ii[None, :], 0.0, -1e30)
    cst[:N, N:2 * N] = np.where(ii[:, None] <= ii[None, :], 0.0, -1e30)

    in_maps = []
    for core in range(NCORES):
        b, g = divmod(core, GROUPS)
        wts_c = wts.copy()
        sel = np.zeros((P, C), np.float32)
        toff = C * g
        sel[toff:toff + C, :] = np.eye(C, dtype=np.float32)
        wts_c[:, SEL_O:SEL_O + C] = sel
        m = {"wts_d": wts_c, "cst_d": cst, "seq_b": seq[b]}
        qs = np.zeros((RT, D), np.float32)
        j0 = RT * g + (C - 1)
        src = seq[b, j0:min(j0 + RT, L)]
        qs[:len(src)] = src
        m["seq_q"] = qs
        in_maps.append(m)
    return in_maps


def kernel(**inputs):
    if "nc" not in _CACHE:
        _CACHE["nc"] = _build()
    nc = _CACHE["nc"]
    in_maps = _host_prep(inputs)
    trace = bool(int(os.environ.get("KERNEL_TRACE", "0")))
    if trace:
        try:
            from antenv.axon_hooks import get_axon_ntff_profile_hook  # noqa: F401
        except ImportError:
            trace = _install_ntff_hook()
    res = run_bass_kernel_spmd(
        nc, in_maps, core_ids=list(range(NCORES)), trace=trace)
    LAST_PERF.clear()
    LAST_PERF.update(dict(
        exec_time_ns=res.exec_time_ns,
        mean_exec_time_ns=res.mean_exec_time_ns,
        profile_json=res.profile_json,
        trace=res.instructions_and_trace[1] if res.instructions_and_trace else None,
    ))
    final = np.zeros((B, L, D), np.float32)
    for core in range(NCORES):
        b, g = divmod(core, GROUPS)
        j0 = RT * g + (C - 1)
        n = min(RT, L - j0)
        final[b, j0:j0 + n] = res.results[core]["out"][:n]
    return final


# revision 21
# speedup vs baseline: 1.9339x; 1.0188x over previous
"""Titans NeuralMemory forward on 8 Trainium2 NeuronCores.

Decomposition (validated vs reference in fp64/numpy):
  - Per-chunk MLP-loss gradients are rank-16: g_i(s) = l_i(s)^T r_i(s) with
    l/r factors [16, 256] from a batched forward/backward pass with the
    shared base weights.
  - The two associative scans have scalar per-chunk coefficients, so their
    composition is a lower-triangular [64, 64] matrix T = L_D @ L_A built
    stably via exp of cumulative log-sigmoid differences.
  - Retrieval never materializes fast weights: per layer,
      X_{i+1} = silu(X_i @ W_i + (X_i @ L_i^T * M) @ R_i),
    where M[r, j] = T[chunk(r), chunk(j)] expands T blockwise.

Sharding: 8 cores = 2 batch rows x 4 retrieve row-groups of 256 rows.
Each core redundantly runs the store phase for its batch row and computes
its own 256 retrieve rows; no collectives. Matmuls in fp32r (full PE rate).

v2 perf restructure vs v1:
  - The [64]-chunk cumulative-gate row broadcast is done on-chip with a
    PE transpose + K=2 ones-matmul instead of a DRAM round-trip whose
    4-byte-element DMAs took ~40us each and stalled the engine FIFOs.
  - Scalar activations are emitted grouped by activation table
    (ln/exp -> sigmoid -> ln/exp -> silu -> ln/exp, with table-free
    Square/Copy in between): 5 ACT_TABLE_LOADs instead of 28.
  - Retrieve silu is a single scalar Silu op (no sigmoid+vector mul).
  - PSUM->SBUF copies are split between scalar (ACT Copy) and vector.
  - Input DMAs reordered (seq first, small weight tail before big blob).
"""
import os
import numpy as np

import concourse.bass as bass
import concourse.tile as tile
from concourse import bacc, mybir
from concourse.bass_utils import run_bass_kernel_spmd

AF = mybir.ActivationFunctionType
ALU = mybir.AluOpType
FP32 = mybir.dt.float32
FP32R = mybir.dt.float32r

B, L, D, C, DEPTH = 2, 1024, 256, 16, 4
N = L // C          # 64 chunks
P = 128
EPS = 1.1920929e-07
NCORES = 8
GROUPS = 4
RT = L // GROUPS    # 256 retrieve rows per core

# weight-blob layout (fp32r, per-partition fp32 word offsets)
WQ_O, WKV_O = 0, 512
W_O = WKV_O + 1024
WT_O = W_O + 2048
WP_O = WT_O + 1536
IDR_O = WP_O + 8
UT_O = IDR_O + 128
NUT_O = UT_O + 128
SEL_O = NUT_O + 128
BSEL_O = SEL_O + 16
WTS_SZ = BSEL_O + 256

_CACHE = {}
LAST_PERF = {}


def _install_ntff_hook():
    """The agent image's antenv lacks axon_hooks; synthesize it so
    run_bass_kernel_spmd's trace=True path can reach the NTFF ctypes hook."""
    import sys
    import types
    try:
        from trn_agent_boot.trn_boot import _ntff_profile_via_ctypes
        hook = _ntff_profile_via_ctypes("/opt/axon/libaxon_pjrt.so")
    except Exception:
        return False
    if hook is None:
        return False
    mod = types.ModuleType("antenv.axon_hooks")
    mod.get_axon_ntff_profile_hook = lambda: hook
    mod.set_axon_ntff_profile_hook = lambda h: None
    sys.modules["antenv.axon_hooks"] = mod
    return True


def _patch_act_tables():
    """Restrict the activation-function table sets the compiler can pick so
    Ln+Exp land in one combined table (the default chooser picks minimal
    sets, forcing a 1.3us ACT_TABLE_LOAD on every Ln<->Exp transition).
    Patches both the bass placement pass and walrus (env var) so the
    emitted act_func_set_ids stay consistent."""
    if _CACHE.get("act_patched"):
        return
    try:
        import json as _json
        from concourse import hw_specs
        from neuronxcc.driver.Job import Job
        from neuronxcc.driver.jobs.support.FindActInfo import findActInfoFile
        src = findActInfoFile(Job.getPackageDir(), "gen3")
        info = _json.load(open(src))
        keep = {"natural_log_exp_and_others", "sigmoid_and_others",
                "silu_and_others"}
        info["act_func_sets"] = [e for e in info["act_func_sets"]
                                 if e["name"] in keep]
        assert len(info["act_func_sets"]) == 3
        path = "/tmp/bass_act_info_filtered.json"
        with open(path, "w") as f:
            _json.dump(info, f)
        tables = {
            e["name"]: {mybir.ActivationFunctionType.from_pwp(v)
                        for v in e["act"].keys()}
            for e in info["act_func_sets"]
        }

        def patched(module_arch):
            return tables

        hw_specs.get_activation_tables = patched
        bacc.get_activation_tables = patched
        os.environ["BASS_ACT_ROOT_JSON_PATH"] = path
        _CACHE["act_patched"] = True
    except Exception:
        pass


def _build():
    if os.environ.get("KERNEL_ACT_PATCH", "0") == "1":
        _patch_act_tables()
    nc = bacc.Bacc("TRN2", target_bir_lowering=False)

    seq_b = nc.dram_tensor("seq_b", [L, D], FP32, kind="ExternalInput")
    seq_q = nc.dram_tensor("seq_q", [RT, D], FP32, kind="ExternalInput")
    wts_d = nc.dram_tensor("wts_d", [P, WTS_SZ], FP32R, kind="ExternalInput")
    cst_d = nc.dram_tensor("cst_d", [P, 2 * N], FP32, kind="ExternalInput")
    out_d = nc.dram_tensor("out", [RT, D], FP32, kind="ExternalOutput")

    with tile.TileContext(nc) as tc:
        with (
            tc.tile_pool(name="big", bufs=1) as big,
            tc.tile_pool(name="rot", bufs=3) as rot,
            tc.tile_pool(name="pmm", bufs=4, space="PSUM") as pmm,
            tc.tile_pool(name="psc", bufs=2, space="PSUM") as psc,
            tc.tile_pool(name="ptr", bufs=2, space="PSUM") as ptr,
        ):
            # ---------------- bulk loads (seq halves first, small wts next) --
            sq8 = big.tile([P, 8, D], FP32, tag="sq8")
            sqv = seq_b[:].rearrange("(i p) d -> p i d", p=P)
            nc.sync.dma_start(sq8[:, 0:4, :], sqv[:, 0:4, :])
            nc.sync.dma_start(sq8[:, 4:8, :], sqv[:, 4:8, :])
            wts = big.tile([P, WTS_SZ], FP32R)
            nc.sync.dma_start(wts[:, IDR_O:WTS_SZ], wts_d[:, IDR_O:WTS_SZ])
            qs2 = big.tile([P, 2, D], FP32, tag="qs2")
            nc.sync.dma_start(qs2, seq_q[:].rearrange("(i p) d -> p i d", p=P))
            nc.sync.dma_start(wts[:, 0:IDR_O], wts_d[:, 0:IDR_O])
            cst = big.tile([P, 2 * N], FP32)
            nc.sync.dma_start(cst, cst_d[:])

            wq_sb = wts[:, WQ_O:WQ_O + 512].rearrange("p (k m) -> p k m", k=2)
            wkv_sb = wts[:, WKV_O:WKV_O + 1024].rearrange("p (k m) -> p k m", k=2)
            w_sb = wts[:, W_O:W_O + 2048].rearrange(
                "p (l k m) -> p l k m", l=4, k=2)
            wt_sb = wts[:, WT_O:WT_O + 1536].rearrange(
                "p (l k m) -> p l k m", l=3, k=2)
            wp_sb = wts[:, WP_O:WP_O + 8].rearrange("p (k m) -> p k m", k=2)
            identR = wts[:, IDR_O:IDR_O + 128]
            ut_sb = wts[:, UT_O:UT_O + 128]
            nut_sb = wts[:, NUT_O:NUT_O + 128]
            sel_sb = wts[:, SEL_O:SEL_O + 16]
            bsel_sb = wts[:, BSEL_O:BSEL_O + 256]
            mls_sb = cst[:, 0:N]
            mut_sb = cst[:, N:2 * N]

            eps_sb = big.tile([P, 1], FP32)
            nc.vector.memset(eps_sb, EPS)
            # pin the ln/exp act table before the Squares pick another set
            dummy = big.tile([P, 1], FP32)
            nc.scalar.activation(dummy, eps_sb, AF.Ln)

            # ---------------- rmsnorms: squares on vector, rstd on scalar --
            xs = [sq8[:, i, :] for i in range(8)] + [qs2[:, i, :] for i in range(2)]
            msb = big.tile([P, 10], FP32)
            for i, x in enumerate(xs):
                scr_a = rot.tile([P, D], FP32, tag="rms_scr", bufs=2)
                nc.scalar.activation(scr_a, x, AF.Square,
                                     accum_out=msb[:, i:i + 1])
            lnv = big.tile([P, 10], FP32)
            rstd = big.tile([P, 10], FP32)
            for a, b in ((0, 4), (4, 8), (8, 10)):
                nc.scalar.activation(lnv[:, a:b], msb[:, a:b], AF.Ln,
                                     scale=1.0 / D, bias=eps_sb)
                nc.scalar.activation(rstd[:, a:b], lnv[:, a:b], AF.Exp,
                                     scale=-0.5)
            sn = [big.tile([P, D], FP32R, name=f"sn{i}") for i in range(8)]
            rq = [big.tile([P, D], FP32R, name=f"rq{i}") for i in range(2)]
            for i in range(8):
                nc.vector.tensor_scalar_mul(sn[i], xs[i], rstd[:, i:i + 1])
            for i in range(2):
                nc.vector.tensor_scalar_mul(rq[i], xs[8 + i], rstd[:, 8 + i:9 + i])

            # ---------------- transposes: snT, rqT ----------------
            snT = [big.tile([P, L], FP32R, name=f"snT{k}", tag=f"snT{k}")
                   for k in range(2)]
            for grp in range(2):
                for ko in range(2):
                    tp = ptr.tile([P, 512], FP32R, tag="tr")
                    for ii in range(4):
                        i = grp * 4 + ii
                        nc.tensor.transpose(
                            tp[:, ii * P:(ii + 1) * P],
                            sn[i][:, ko * P:(ko + 1) * P], identR)
                    eng = nc.vector if (grp + ko) % 2 == 0 else nc.scalar
                    if eng is nc.vector:
                        nc.vector.tensor_copy(
                            snT[ko][:, grp * 512:(grp + 1) * 512], tp)
                    else:
                        nc.scalar.copy(
                            snT[ko][:, grp * 512:(grp + 1) * 512], tp)
            rqT = [big.tile([P, RT], FP32R, name=f"rqT{k}") for k in range(2)]
            for ko in range(2):
                tp = ptr.tile([P, 512], FP32R, tag="tr")
                for rt in range(2):
                    nc.tensor.transpose(
                        tp[:, rt * P:(rt + 1) * P],
                        rq[rt][:, ko * P:(ko + 1) * P], identR)
                nc.scalar.copy(rqT[ko], tp[:, 0:RT])

            # ---------------- chunk sums -> gate logits ----------------
            cmT = big.tile([P, 2, N], FP32R)
            with nc.allow_low_precision(reason="fp32r rounding of fp32 accum"):
                for ko in range(2):
                    nc.vector.reduce_sum(
                        cmT[:, ko, :],
                        snT[ko].rearrange("p (n c) -> p n c", c=C),
                        axis=mybir.AxisListType.X)

            zp = ptr.tile([N, 4], FP32, tag="tr")
            for ko in range(2):
                nc.tensor.matmul(zp, cmT[:, ko, :], wp_sb[:, ko, :],
                                 start=(ko == 0), stop=(ko == 1))
            # gate sigmoids (head of the sigmoid-table block; fwd sigmoids
            # follow with only table-free Copies in between)
            sg = big.tile([P, 3], FP32)
            nc.vector.memset(sg, 0.0)
            nc.scalar.activation(sg[:N, 0:1], zp[:, 1:2], AF.Sigmoid)
            nc.scalar.activation(sg[:N, 1:2], zp[:, 2:3], AF.Sigmoid, scale=-1.0)
            nc.scalar.activation(sg[:N, 2:3], zp[:, 0:1], AF.Sigmoid)

            # ---------------- kv projection ----------------
            kT = [big.tile([P, L], FP32R, name=f"kT{k}") for k in range(2)]
            vT = [big.tile([P, L], FP32, name=f"vT{k}") for k in range(2)]
            for ko4 in range(4):
                dest = kT[ko4] if ko4 < 2 else vT[ko4 - 2]
                for rc in range(2):
                    sl = slice(rc * 512, (rc + 1) * 512)
                    mm = pmm.tile([P, 512], FP32, tag="mm")
                    for ki in range(2):
                        nc.tensor.matmul(
                            mm, wkv_sb[:, ki, ko4 * P:(ko4 + 1) * P],
                            snT[ki][:, sl], start=(ki == 0), stop=(ki == 1))
                    nc.scalar.copy(dest[:, sl], mm)

            # retrieve query projection (early: fills PE between kv and fwd)
            XTa = [big.tile([P, RT], FP32R, name=f"XTa{k}") for k in range(2)]
            for mo in range(2):
                sc = psc.tile([P, RT], FP32, tag="sc")
                for ki in range(2):
                    nc.tensor.matmul(sc, wq_sb[:, ki, mo * P:(mo + 1) * P],
                                     rqT[ki], start=(ki == 0), stop=(ki == 1))
                nc.vector.tensor_copy(XTa[mo], sc)

            # ---------------- forward MLP ----------------
            Lf = [kT]
            dsT = []
            for i in range(3):
                a_next = [big.tile([P, L], FP32R, name=f"aT{i+1}_{k}")
                          for k in range(2)]
                ds_i = [big.tile([P, L], FP32, name=f"dsT{i}_{k}")
                        for k in range(2)]
                for mo in range(2):
                    for rc in range(2):
                        sl = slice(rc * 512, (rc + 1) * 512)
                        mm = pmm.tile([P, 512], FP32, tag="mm")
                        for ki in range(2):
                            nc.tensor.matmul(
                                mm, w_sb[:, i, ki, mo * P:(mo + 1) * P],
                                Lf[i][ki][:, sl],
                                start=(ki == 0), stop=(ki == 1))
                        sgt = rot.tile([P, 512], FP32, tag="sgt", bufs=2)
                        nc.scalar.activation(sgt, mm, AF.Sigmoid)
                        nc.vector.tensor_mul(a_next[mo][:, sl], mm, sgt)
                        # ds = sig * (1 + h - a); final mult off-path on gpsimd
                        t2 = rot.tile([P, 512], FP32, tag="t2", bufs=2)
                        nc.vector.scalar_tensor_tensor(
                            out=t2, in0=mm, scalar=1.0, in1=a_next[mo][:, sl],
                            op0=ALU.add, op1=ALU.subtract)
                        nc.gpsimd.tensor_mul(ds_i[mo][:, sl], sgt, t2)
                Lf.append(a_next)
                dsT.append(ds_i)

            # ---------------- pred + gg3 ----------------
            ggA = [big.tile([P, L], FP32R, name=f"ggA{k}", tag=f"snT{k}")
                   for k in range(2)]
            ggB = [big.tile([P, L], FP32R, name="ggB0", tag="sq8"),
                   big.tile([P, L], FP32R, name="ggB1", tag="qs2")]
            for mo in range(2):
                for rc in range(2):
                    sl = slice(rc * 512, (rc + 1) * 512)
                    mm = pmm.tile([P, 512], FP32, tag="mm")
                    for ki in range(2):
                        nc.tensor.matmul(
                            mm, w_sb[:, 3, ki, mo * P:(mo + 1) * P],
                            Lf[3][ki][:, sl], start=(ki == 0), stop=(ki == 1))
                    nc.vector.tensor_sub(ggA[mo][:, sl], vT[mo][:, sl], mm)

            # ---------------- T pipeline (gates -> masks), on-chip ------
            # scalar block: ln of gate sigmoids, then the two exps (same
            # natural_log_exp table, after all fwd sigmoids in the stream)
            lg = big.tile([P, 3], FP32)
            nc.vector.memset(lg, 0.0)
            nc.scalar.activation(lg[:N, :], sg[:N, :], AF.Ln)
            lgr = big.tile([P, 2], FP32R)
            nc.vector.tensor_copy(lgr, lg[:, 0:2])
            cacc_p = ptr.tile([P, 2], FP32, tag="tr")
            nc.tensor.matmul(cacc_p, ut_sb, lgr, start=True, stop=True)
            cacc = big.tile([P, 2], FP32)
            nc.vector.tensor_copy(cacc, cacc_p)
            nacc_p = ptr.tile([P, 2], FP32, tag="tr")
            nc.tensor.matmul(nacc_p, nut_sb, lgr, start=True, stop=True)
            nacc = big.tile([P, 2], FP32)
            nc.vector.tensor_copy(nacc, nacc_p)

            # stage [NACC0 + ln(2 lr / D) | CACC1]; transpose to a 2-row
            # tile and broadcast across partitions with a K=2 ones-matmul.
            # Folding the surprise scale (2/D)*lr_s into T's s-columns lets
            # gg3 = v - pred with no broadcast dependency.
            stage = big.tile([P, 2], FP32)
            nc.vector.scalar_tensor_tensor(
                out=stage[:, 0:1], in0=nacc[:, 0:1],
                scalar=float(np.log(2.0 / D)), in1=lg[:, 2:3],
                op0=ALU.add, op1=ALU.add)
            nc.vector.tensor_copy(stage[:, 1:2], cacc[:, 1:2])
            stg128 = big.tile([P, P], FP32)
            nc.vector.memset(stg128, 0.0)
            nc.vector.tensor_copy(stg128[:, 0:2], stage)
            stg128r = big.tile([P, P], FP32R)
            nc.vector.tensor_copy(stg128r, stg128)
            tp2 = ptr.tile([P, 512], FP32R, tag="tr")
            nc.tensor.transpose(tp2[:, 0:P], stg128r, identR)
            st_sb = big.tile([P, P], FP32R)
            nc.vector.tensor_copy(st_sb, tp2[:, 0:P])
            bc = ptr.tile([P, 2, N], FP32, tag="tr")
            nc.tensor.matmul(bc[:, 0, :], bsel_sb[:, 0:P], st_sb[:, 0:N],
                             start=True, stop=True)
            nc.tensor.matmul(bc[:, 1, :], bsel_sb[:, P:2 * P], st_sb[:, 0:N],
                             start=True, stop=True)

            la = big.tile([P, N], FP32R)
            tmp1 = big.tile([P, N], FP32)
            nc.vector.scalar_tensor_tensor(
                out=tmp1, in0=bc[:, 0, :], scalar=cacc[:, 0:1], in1=mls_sb,
                op0=ALU.add, op1=ALU.add)
            nc.scalar.activation(la, tmp1, AF.Exp)
            ldt = big.tile([P, N], FP32R)
            tmp2 = big.tile([P, N], FP32)
            nc.vector.scalar_tensor_tensor(
                out=tmp2, in0=bc[:, 1, :], scalar=nacc[:, 1:2], in1=mut_sb,
                op0=ALU.add, op1=ALU.add)
            nc.scalar.activation(ldt, tmp2, AF.Exp)

            tt_p = ptr.tile([N, N], FP32, tag="tr")
            nc.tensor.matmul(tt_p, ldt, la, start=True, stop=True)
            ttile = big.tile([P, N], FP32)
            nc.vector.memset(ttile, 0.0)
            nc.vector.tensor_copy(ttile[:N], tt_p)

            # maskbx_k[j, r] = T[toff + r//16, s(j)]  (expanded x16 in r)
            maskbx = []
            for k in range(8):
                ttx = rot.tile([P, P], FP32R, tag="ttx", bufs=2)
                nc.gpsimd.tensor_copy(
                    ttx[:N],
                    ttile[:N, k * 8:(k + 1) * 8, None].to_broadcast([N, 8, C]))
                mb_p = ptr.tile([P, C], FP32, tag="tr")
                nc.tensor.matmul(mb_p, ttx[:N], sel_sb[:N], start=True,
                                 stop=True)
                mb = rot.tile([P, C], FP32, tag="mb", bufs=2)
                nc.vector.tensor_copy(mb, mb_p)
                mbx = big.tile([P, RT], FP32, name=f"maskbx{k}")
                nc.gpsimd.tensor_copy(
                    mbx.rearrange("p (n c) -> p n c", c=C),
                    mb[:, :, None].to_broadcast([P, C, C]))
                maskbx.append(mbx)

            # ---------------- R factors + backward ----------------
            Rf = {i: [big.tile([P, D], FP32R, name=f"Rf{i}_{jt}")
                      for jt in range(8)] for i in range(4)}

            def emit_R(layer, src):
                for jt in range(8):
                    tp = ptr.tile([P, 512], FP32R, tag="tr")
                    for mo in range(2):
                        nc.tensor.transpose(
                            tp[:, mo * P:(mo + 1) * P],
                            src[mo][:, jt * P:(jt + 1) * P], identR)
                    if jt % 2 == 0:
                        nc.vector.tensor_copy(Rf[layer][jt], tp[:, 0:D])
                    else:
                        nc.scalar.copy(Rf[layer][jt], tp[:, 0:D])

            emit_R(3, ggA)
            gg_cur, gg_next = ggA, ggB
            for i in (3, 2, 1):
                for mo in range(2):
                    for rc in range(2):
                        sl = slice(rc * 512, (rc + 1) * 512)
                        mm = pmm.tile([P, 512], FP32, tag="mm")
                        for ki in range(2):
                            nc.tensor.matmul(
                                mm, wt_sb[:, i - 1, ki, mo * P:(mo + 1) * P],
                                gg_cur[ki][:, sl],
                                start=(ki == 0), stop=(ki == 1))
                        nc.vector.tensor_mul(
                            gg_next[mo][:, sl], mm, dsT[i - 1][mo][:, sl])
                emit_R(i - 1, gg_next)
                gg_cur, gg_next = gg_next, gg_cur

            # ---------------- retrieve ----------------
            XTb = [big.tile([P, RT], FP32R, name=f"XTb{k}") for k in range(2)]
            XTin, XTout = XTa, XTb
            X4T = [big.tile([P, RT], FP32R, name=f"X4T{k}") for k in range(2)]
            for i in range(4):
                msc = []
                for jt in range(8):
                    sc = psc.tile([P, RT], FP32, tag="sc")
                    for ki in range(2):
                        nc.tensor.matmul(
                            sc, Lf[i][ki][:, jt * P:(jt + 1) * P], XTin[ki],
                            start=(ki == 0), stop=(ki == 1))
                    m = rot.tile([P, RT], FP32R, tag="msc", bufs=8)
                    nc.vector.tensor_mul(m, sc, maskbx[jt])
                    msc.append(m)
                for mo in range(2):
                    yw = ptr.tile([P, 512], FP32, tag="tr")
                    y = yw[:, 0:RT]
                    for ki in range(2):
                        nc.tensor.matmul(
                            y, w_sb[:, i, ki, mo * P:(mo + 1) * P], XTin[ki],
                            start=(ki == 0), stop=False)
                    for jt in range(8):
                        nc.tensor.matmul(
                            y, Rf[i][jt][:, mo * P:(mo + 1) * P], msc[jt],
                            start=False, stop=(jt == 7))
                    if i < 3:
                        nc.scalar.activation(XTout[mo], y, AF.Silu)
                    else:
                        nc.vector.tensor_copy(X4T[mo], y)
                XTin, XTout = XTout, XTin

            # ---------------- postnorm + output ----------------
            x4 = []
            pms = big.tile([P, 2], FP32)
            for rt in range(2):
                tp = ptr.tile([P, 512], FP32R, tag="tr")
                for mo in range(2):
                    nc.tensor.transpose(
                        tp[:, mo * P:(mo + 1) * P],
                        X4T[mo][:, rt * P:(rt + 1) * P], identR)
                x4t = rot.tile([P, D], FP32, tag="x4", bufs=2)
                nc.vector.tensor_copy(x4t, tp[:, 0:D])
                x4.append(x4t)
                scr_a = rot.tile([P, D], FP32, tag="rms_scr", bufs=2)
                nc.scalar.activation(scr_a, x4t, AF.Square,
                                     accum_out=pms[:, rt:rt + 1])
            pln = big.tile([P, 2], FP32)
            nc.scalar.activation(pln, pms, AF.Ln, scale=1.0 / D, bias=eps_sb)
            prs = big.tile([P, 2], FP32)
            nc.scalar.activation(prs, pln, AF.Exp, scale=-0.5)
            for rt in range(2):
                o = rot.tile([P, D], FP32, tag="osb", bufs=2)
                nc.vector.tensor_scalar_mul(o, x4[rt], prs[:, rt:rt + 1])
                nc.sync.dma_start(out_d[rt * P:(rt + 1) * P, :], o)

    nc.compile()
    return nc


def _host_prep(inputs):
    seq = np.ascontiguousarray(np.asarray(inputs["seq"], dtype=np.float32))
    Wq = np.asarray(inputs["Wq"], dtype=np.float32)
    Wkv = np.asarray(inputs["Wkv"], dtype=np.float32)
    Ws = [np.asarray(inputs[f"W{i}"], dtype=np.float32) for i in range(4)]
    wa = np.asarray(inputs["w_adapt"], dtype=np.float32)
    wm = np.asarray(inputs["w_mom"], dtype=np.float32)
    wd = np.asarray(inputs["w_decay"], dtype=np.float32)

    def kxm(w):  # [K, M] -> [128, (K/128)*M]
        return w.reshape(w.shape[0] // P, P, w.shape[1]).transpose(1, 0, 2) \
            .reshape(P, -1)

    ii = np.arange(N)
    tri = np.triu(np.ones((N, N), np.float32))
    wpack = np.zeros((D, 4), np.float32)
    wpack[:, 0] = wa
    wpack[:, 1] = wm
    wpack[:, 2] = wd
    wpack *= (1.0 / C)

    wts = np.zeros((P, WTS_SZ), np.float32)
    wts[:, WQ_O:WQ_O + 512] = kxm(Wq)
    wts[:, WKV_O:WKV_O + 1024] = kxm(Wkv)
    w_all = np.stack(Ws).reshape(4, 2, P, D).transpose(2, 0, 1, 3)
    wts[:, W_O:W_O + 2048] = w_all.reshape(P, -1)
    wt_all = np.stack([Ws[1].T, Ws[2].T, Ws[3].T]) \
        .reshape(3, 2, P, D).transpose(2, 0, 1, 3)
    wts[:, WT_O:WT_O + 1536] = wt_all.reshape(P, -1)
    wts[:, WP_O:WP_O + 8] = kxm(wpack)
    wts[:, IDR_O:IDR_O + 128] = np.eye(P, dtype=np.float32)
    wts[:N, UT_O:UT_O + N] = tri
    wts[:N, NUT_O:NUT_O + N] = -tri
    wts[0, BSEL_O:BSEL_O + P] = 1.0
    wts[1, BSEL_O + P:BSEL_O + 2 * P] = 1.0

    cst = np.full((P, 2 * N), -1e30, np.float32)
    cst[:N, 0:N] = np.where(ii[:, None] >= ii[None, :], 0.0, -1e30)
    cst[:N, N:2 * N] = np.where(ii[:, None] <= ii[None, :], 0.0, -1e30)

    in_maps = []
    for core in range(NCORES):
        b, g = divmod(core, GROUPS)
        wts_c = wts.copy()
        sel = np.zeros((P, C), np.float32)
        toff = C * g
        sel[toff:toff + C, :] = np.eye(C, dtype=np.float32)
        wts_c[:, SEL_O:SEL_O + C] = sel
        m = {"wts_d": wts_c, "cst_d": cst, "seq_b": seq[b]}
        qs = np.zeros((RT, D), np.float32)
        j0 = RT * g + (C - 1)
        src = seq[b, j0:min(j0 + RT, L)]
        qs[:len(src)] = src
        m["seq_q"] = qs
        in_maps.append(m)
    return in_maps


def kernel(**inputs):
    if "nc" not in _CACHE:
        _CACHE["nc"] = _build()
    nc = _CACHE["nc"]
    in_maps = _host_prep(inputs)
    trace = bool(int(os.environ.get("KERNEL_TRACE", "0")))
    if trace:
        try:
            from antenv.axon_hooks import get_axon_ntff_profile_hook  # noqa: F401
        except ImportError:
            trace = _install_ntff_hook()
    res = run_bass_kernel_spmd(
        nc, in_maps, core_ids=list(range(NCORES)), trace=trace)
    LAST_PERF.clear()
    LAST_PERF.update(dict(
        exec_time_ns=res.exec_time_ns,
        mean_exec_time_ns=res.mean_exec_time_ns,
        profile_json=res.profile_json,
        trace=res.instructions_and_trace[1] if res.instructions_and_trace else None,
    ))
    final = np.zeros((B, L, D), np.float32)
    for core in range(NCORES):
        b, g = divmod(core, GROUPS)
        j0 = RT * g + (C - 1)
        n = min(RT, L - j0)
        final[b, j0:j0 + n] = res.results[core]["out"][:n]
    return final
